# revision 51
# baseline (speedup 1.0000x reference)
"""CenterPooling (CornerNet) Trainium2 kernel — 8 NeuronCores.

Sharding: 8 cores = 4 batches x 2 H-halves.  Each core gets a host-padded
input slab (3 halo rows each side, zero W-pad columns).

Key algebraic simplifications:
 - cummax(reverse) then cummax(forward) along an axis == global max along
   that axis, broadcast.  So the up branch only needs per-row maxes over W
   ([C, H]) and the down branch per-column maxes over H ([C, W]).
 - BN (eval mode) folds into conv weights/bias on the host; BN scale > 0 so
   max-reduction commutes with the affine+ReLU epilogue.
 - The merge conv's input is rank-structured: updown[c,h,w] = u[c,h] + d[c,w],
   so the 3x3 merge conv SEPARATES into tiny 1-D convs: an h-conv of u
   ([C, H] -> A(o,h), with 3 w-boundary classes of kx-summed weights) plus a
   w-conv of d ([C, W] -> B(o,w), with h-boundary corrections applied
   data-driven via per-row selector vectors).
 - Down-branch col-max needs a cross-half combine: pairwise AllReduce-max of
   a tiny [256, 128] tile.
 - H-pad semantics at the global top/bottom are handled data-driven (SPMD
   uniform program): a validity mask zeroes invalid u rows, and a per-row
   -1e30 bias on relu1 clamps out-of-range rows to the zero-pad value.

Precision: the up/down/c1 convs run as fp8e4 (TRN FP8_EXP4, max 240)
DoubleRow matmuls; the output conv block (c2) runs 8 of its 9 taps bf16
plus the (0,0) tap as one fp8 DoubleRow MM per block (full-fp8 c2 measures
3.6e-2 — past the 2e-2 gate; this 1-of-9 hybrid measures 1.53e-2), with
the bf16 weights pre-scaled so both paths share one fp32 PSUM accumulation
and the common scale folded into the output ACT.  Output is stored bf16
and upcast on the host (~0.1% extra L2, halves store traffic).

Schedule notes:
 - Input DMA descriptors are issued from BOTH hardware DGE queues (sync +
   scalar) so descriptor generation (~0.7us each) does not serialize the
   startup: weights on sync, the x8 slab on scalar; weights are packed
   cot-major so the first conv group gates on half the weight bytes.
 - The up conv computes only its 64 own rows (16 blocks): the 4 halo rows
   it used to recompute ride the pairwise AllReduce as 4 extra "slot"
   columns (each core sends its half's two slots and -3e38 elsewhere, so
   the max IS the partner's value, and hv masks the rows that don't apply).
 - ufin/umask are produced incrementally per up-conv block; dfin/dpad, the
   B conv, the first half of the A conv, and the first three relu1 blocks
   are woven into the up-conv stream (the woven c1 MMs use a dedicated
   psum tag so the up-conv bank rotation never couples to relu1 STTs).
 - relu1 assembly is one c1 MM -> one STT -> one ACT per 4-row block-cot:
   the combined bias tensor btaf = bt + afull + edge fixups is precomputed
   on vector one step ahead, off the critical chain.
 - The remaining relu1 blocks interleave with c2 blocks so the PE always
   has a c2 block queued; the last c2 block is split 2+2 rows to shorten
   the final ACT+store tail.
"""

import sys

sys.path.insert(0, "/opt/trn_rl_repo")

import numpy as np
import ml_dtypes

import concourse.bacc as bacc
import concourse.tile as tile
import concourse.bass as bass
from concourse import mybir, bass_utils

BF16 = mybir.dt.bfloat16
FP8 = mybir.dt.float8e4
F32 = mybir.dt.float32
NP_BF16 = ml_dtypes.bfloat16
NP_FP8 = ml_dtypes.float8_e4m3  # IEEE e4m3, max 240 == TRN FP8_EXP4

N_CORES = 8
B, CIN, C, H, W = 4, 256, 256, 128, 128
G = 3            # halo rows on each side of the 64 owned rows
HS = 64 + 2 * G  # 70 slab rows
WP = W + 2       # 130 (zero-pad col on each side) — bf16 r1 slab
WPX = 144        # fp8 x slab width: 16B-aligned row and plane strides
EPS = 1e-5
NEG = -1e30
QCLIP = 224.0    # fp8 absmax target (headroom under the 240 max)

RELU = mybir.ActivationFunctionType.Relu
AX_X = mybir.AxisListType.X
ALU = mybir.AluOpType
DROW = mybir.MatmulPerfMode.DoubleRow


def _mm_group(nc, ps_ap, mms, perf_mode=None):
    n = len(mms)
    for k, (lhsT, rhs) in enumerate(mms):
        nc.tensor.matmul(ps_ap, lhsT, rhs, start=(k == 0), stop=(k == n - 1),
                         perf_mode=perf_mode)


def _conv3_mms8(wtile, x8, s, nr, cot):
    """The 9 (ky,kx) DoubleRow matmuls of a 3x3 conv: output rows s..s+nr-1.

    wtile is [128ci, 2cit, 2cot, 9j, 128co]; the lhsT AP [128, 2, 128] takes
    the cit planes at stride 2*9*128 (16B-aligned as DoubleRow requires)."""
    mms = []
    for ky in range(3):
        for kx in range(3):
            mms.append((wtile[:, :, cot, ky * 3 + kx, :],
                        x8[:, :, s + ky - 1:s + ky - 1 + nr, kx:kx + W]))
    return mms


def _conv3_mms(wtile, src, s, nr, cot, skip00=False):
    """The (ci,ky,kx) bf16 matmuls of a 3x3 conv block (18, or 16 when the
    (0,0) tap is carried by the fp8 path)."""
    mms = []
    for cit in range(2):
        for ky in range(3):
            for kx in range(3):
                if skip00 and ky == 0 and kx == 0:
                    continue
                j = ((ky * 3 + kx) * 2 + cit) * 2 + cot
                mms.append((wtile[:, j, :], src[cit][:, s + ky - 1:s + ky - 1 + nr, kx:kx + W]))
    return mms


def _build(qs):
    nc = bacc.Bacc("TRN2", target_bir_lowering=False, debug=False,
                   num_devices=N_CORES)

    x8_d = nc.dram_tensor("x8", [128, 2, HS, WPX], FP8, kind="ExternalInput")
    # (cit, cot) are OUTERMOST in dram so each per-(cit,cot) DMA is fully
    # contiguous per partition; the first down MM then only gates on the
    # cot=0 halves.
    wup_d = nc.dram_tensor("wup", [2, 2, 128, 9, 128], FP8, kind="ExternalInput")
    wdn_d = nc.dram_tensor("wdn", [2, 2, 128, 9, 128], FP8, kind="ExternalInput")
    wc1_d = nc.dram_tensor("wc1", [2, 128, 2, 128], FP8, kind="ExternalInput")
    wc2_d = nc.dram_tensor("wc2", [128, 36, 128], BF16, kind="ExternalInput")
    wc2f8_d = nc.dram_tensor("wc2f8", [2, 128, 2, 128], FP8, kind="ExternalInput")
    wa_d = nc.dram_tensor("wa", [128, 36, 128], BF16, kind="ExternalInput")
    wb_d = nc.dram_tensor("wb", [128, 36, 128], BF16, kind="ExternalInput")
    bias_d = nc.dram_tensor("biases", [128, 8], F32, kind="ExternalInput")
    hv_d = nc.dram_tensor("hv", [128, HS], F32, kind="ExternalInput")
    # u-halo exchange masks: slots [g62, g63, g64, g65]; each core SENDS the
    # two slots its half owns (mask 1 / bias 0) and contributes -3e38
    # elsewhere, so the pairwise AllReduce-max doubles as the exchange.
    sm_d = nc.dram_tensor("sendm", [128, 4], F32, kind="ExternalInput")
    sb_d = nc.dram_tensor("sendb", [128, 4], F32, kind="ExternalInput")
    pnegb_d = nc.dram_tensor("pnegb", [128, HS], F32, kind="ExternalInput")
    htop_d = nc.dram_tensor("htopneg", [128, HS], F32, kind="ExternalInput")
    hbot_d = nc.dram_tensor("hbotneg", [128, HS], F32, kind="ExternalInput")
    out_d = nc.dram_tensor("out", [2, 128, 64, W], BF16, kind="ExternalOutput")

    with tile.TileContext(nc) as tc:
        with tc.tile_pool(name="const", bufs=1) as constp, \
             tc.tile_pool(name="acts", bufs=1) as actp, \
             tc.tile_pool(name="psum", bufs=6, space="PSUM") as psp, \
             tc.tile_pool(name="ostage", bufs=6) as osp, \
             tc.tile_pool(name="dram", bufs=1, space="DRAM") as dramp:

            # --- input DMA: weights on the sync queue, x8 on the scalar
            # queue (both are hardware DGE engines) so descriptor
            # generation overlaps and the first conv can start sooner.
            # A tiny transfer goes first on each queue to warm the DMA
            # engines' descriptor rings before the critical transfers.
            biases = constp.tile([128, 8], F32)
            nc.sync.dma_start(biases[:, :], bias_d.ap())
            wdn = constp.tile([128, 2, 2, 9, 128], FP8)
            for cot in range(2):
                for cit in range(2):
                    nc.sync.dma_start(wdn[:, cit, cot, :, :],
                                      wdn_d.ap()[cit, cot, :, :, :])

            x8 = actp.tile([128, 2, HS, WPX], FP8, name="x8")
            # rows 0,1 are never read (the first conv block reads from row
            # 2); a 1-row probe warms the DMA engines, then rows 2..6 of
            # both cits land first (feeds the first down block), then
            # 16-row chunks
            nc.scalar.dma_start(x8[:, 0, 2:3, :], x8_d.ap()[:, 0, 2:3, :])
            nc.scalar.dma_start(x8[:, 0, 3:7, :], x8_d.ap()[:, 0, 3:7, :])
            nc.scalar.dma_start(x8[:, 1, 2:7, :], x8_d.ap()[:, 1, 2:7, :])
            row_chunks = [(7, 23), (23, 39), (39, 55), (55, HS)]
            for r0, r1_ in row_chunks:
                for cit in range(2):
                    nc.scalar.dma_start(x8[:, cit, r0:r1_, :], x8_d.ap()[:, cit, r0:r1_, :])

            wup = constp.tile([128, 2, 2, 9, 128], FP8)
            for cot in range(2):
                for cit in range(2):
                    nc.sync.dma_start(wup[:, cit, cot, :, :],
                                      wup_d.ap()[cit, cot, :, :, :])
            wc1 = constp.tile([128, 2, 2, 128], FP8)
            for cit in range(2):
                nc.sync.dma_start(wc1[:, cit, :, :], wc1_d.ap()[cit, :, :, :])
            wc2f8 = constp.tile([128, 2, 2, 128], FP8)
            for cit in range(2):
                nc.sync.dma_start(wc2f8[:, cit, :, :], wc2f8_d.ap()[cit, :, :, :])
            hv = constp.tile([128, HS], F32)
            nc.sync.dma_start(hv[:, :], hv_d.ap())
            wa = constp.tile([128, 36, 128], BF16)
            nc.sync.dma_start(wa[:, :, :], wa_d.ap())
            wb = constp.tile([128, 36, 128], BF16)
            nc.sync.dma_start(wb[:, :, :], wb_d.ap())
            pnegb = constp.tile([128, HS], F32)
            nc.sync.dma_start(pnegb[:, :], pnegb_d.ap())
            htopneg = constp.tile([128, HS], F32)
            nc.sync.dma_start(htopneg[:, :], htop_d.ap())
            hbotneg = constp.tile([128, HS], F32)
            nc.sync.dma_start(hbotneg[:, :], hbot_d.ap())
            wc2 = constp.tile([128, 36, 128], BF16)
            nc.sync.dma_start(wc2[:, :, :], wc2_d.ap())
            sendm = constp.tile([128, 4], F32)
            nc.sync.dma_start(sendm[:, :], sm_d.ap())
            sendb = constp.tile([128, 4], F32)
            nc.sync.dma_start(sendb[:, :], sb_d.ap())

            r1 = []
            for cit in range(2):
                t2 = actp.tile([128, HS, WP], BF16, name=f"r1{cit}")
                nc.vector.memset(t2[:, :, 0], 0.0)
                nc.vector.memset(t2[:, :, WP - 1], 0.0)
                r1.append(t2)
            # fp8 copy of relu1 in x8-style layout for the c2 (0,0)-tap
            # DoubleRow matmul (136 = 16B-aligned row stride)
            WF8 = 136
            r1f8 = actp.tile([128, 2, HS, WF8], FP8, name="r1f8")
            for cit in range(2):
                nc.vector.memset(r1f8[:, cit, :, 0], 0.0)
                nc.vector.memset(r1f8[:, cit, :, W + 1], 0.0)

            uraw, ufin, umask, dacc, dmax, dfin = [], [], [], [], [], []
            for cot in range(2):
                uraw.append(actp.tile([128, HS], F32, name=f"uraw{cot}"))
                ufin.append(actp.tile([128, HS], F32, name=f"ufin{cot}"))
                umask.append(actp.tile([128, HS], BF16, name=f"umask{cot}"))
                t = actp.tile([128, W], F32, name=f"dacc{cot}")
                nc.vector.memset(t[:, :], -3e38)
                dacc.append(t)
                dmax.append(actp.tile([128, W], F32, name=f"dmax{cot}"))
                dfin.append(actp.tile([128, W], F32, name=f"dfin{cot}"))

            # ---- down branch: fp8 conv over the 64 owned rows, col-max over H ----
            for i in range(16):
                s = G + 4 * i
                for cot in range(2):
                    ps = psp.tile([128, 4, 128], F32, tag="ps", name="ps_dn", bufs=4)
                    _mm_group(nc, ps[:, :, :], _conv3_mms8(wdn, x8, s, 4, cot),
                              perf_mode=DROW)
                    for rr in range(4):
                        nc.vector.tensor_max(dacc[cot][:, :], dacc[cot][:, :], ps[:, rr, :])

            # pairwise (same-batch) AllReduce-max of the down-branch col-max,
            # fired IMMEDIATELY after the down conv so the scheduler can
            # place the dependent dfin/dpad/B chain mid-up-conv.
            # (values carry the quant scales; both group members match)
            GROUPS = [[0, 1], [2, 3], [4, 5], [6, 7]]
            cc_in = dramp.tile([256, W], F32)
            cc_out = dramp.tile([256, W], F32)
            for cot in range(2):
                nc.sync.dma_start(cc_in[cot * 128:(cot + 1) * 128, :], dacc[cot][:, :])
            nc.gpsimd.collective_compute(
                "AllReduce", ALU.max, replica_groups=GROUPS,
                ins=[cc_in.opt()], outs=[cc_out.opt()])
            for cot in range(2):
                nc.sync.dma_start(dmax[cot][:, :], cc_out[cot * 128:(cot + 1) * 128, :])

            # second, tiny AllReduce doubling as the u-halo exchange: slots
            # [g62, g63, g64, g65] per cot — each core sends its half's two
            # slots, -3e38 elsewhere, so max == the partner's value.
            cc2_in = dramp.tile([256, 4], F32)
            cc2_out = dramp.tile([256, 4], F32)
            ccx = actp.tile([128, 2, 4], F32, name="ccx")
            uex = actp.tile([128, 2, 4], F32, name="uex")

            def emit_cc_fill_and_reduce():
                # slots g62,g63 come from uraw rows 65,66 on half 0;
                # slots g64,g65 from rows 3,4 on half 1
                for cot in range(2):
                    nc.vector.tensor_mul(ccx[:, cot, 0:2], uraw[cot][:, 65:67],
                                         sendm[:, 0:2])
                    nc.vector.tensor_mul(ccx[:, cot, 2:4], uraw[cot][:, 3:5],
                                         sendm[:, 2:4])
                    nc.vector.tensor_add(ccx[:, cot, :], ccx[:, cot, :], sendb[:, :])
                    nc.sync.dma_start(cc2_in[cot * 128:(cot + 1) * 128, :],
                                      ccx[:, cot, :])
                nc.gpsimd.collective_compute(
                    "AllReduce", ALU.max, replica_groups=GROUPS,
                    ins=[cc2_in.opt()], outs=[cc2_out.opt()])
                for cot in range(2):
                    nc.sync.dma_start(uex[:, cot, :], cc2_out[cot * 128:(cot + 1) * 128, :])

            # ---- up branch: fp8 conv over rows [1, 69), row-max over W ----
            # ufin/umask are produced per block; the merge pieces (dfin/dpad,
            # the B conv, the first half of the A conv) and the first relu1
            # blocks are woven INTO the up-conv stream so no dependency chain
            # is exposed when the up conv drains.
            NA = 64 + 2        # A-conv output rows 2..67
            NA1 = 40           # half 1: rows 2..41  (needs umask 1..42)
            NA2 = NA - NA1     # half 2: rows 42..67 (needs umask 41..68)
            dpad = [None, None]
            asb = [[None, None, None], [None, None, None]]
            bt = [[None, None, None], [None, None, None]]
            afull, afdl, afdr = [], [], []
            for cot in range(2):
                afull.append(actp.tile([128, HS], F32, name=f"afull{cot}"))
                afdl.append(actp.tile([128, HS], F32, name=f"afdl{cot}"))
                afdr.append(actp.tile([128, HS], F32, name=f"afdr{cot}"))

            def emit_dfin_dpad():
                for cot in range(2):
                    nc.scalar.activation(dfin[cot][:, :], dmax[cot][:, :], RELU,
                                         bias=biases[:, 2 + cot:3 + cot],
                                         scale=qs["dn"][cot])
                    t = actp.tile([128, WP], BF16, name=f"dpad{cot}")
                    nc.vector.memset(t[:, :], 0.0)
                    nc.vector.tensor_copy(t[:, 1:W + 1], dfin[cot][:, :])
                    dpad[cot] = t

            def emit_uex():
                # exchanged u-halo rows: slots g62,g63 -> slab rows 1,2;
                # slots g64,g65 -> rows 67,68 (hv zeroes the ones that
                # don't apply to this half)
                for cot in range(2):
                    nc.scalar.activation(ufin[cot][:, 1:3], uex[:, cot, 0:2],
                                         RELU, bias=biases[:, cot:cot + 1],
                                         scale=qs["up"][cot])
                    nc.scalar.activation(ufin[cot][:, 67:69], uex[:, cot, 2:4],
                                         RELU, bias=biases[:, cot:cot + 1],
                                         scale=qs["up"][cot])
                    nc.vector.tensor_mul(umask[cot][:, 1:3], ufin[cot][:, 1:3],
                                         hv[:, 1:3])
                    nc.vector.tensor_mul(umask[cot][:, 67:69], ufin[cot][:, 67:69],
                                         hv[:, 67:69])

            def emit_a_half(r0, na):
                # A_cls(o,h): 1-D h-conv of umask with kx-summed merge
                # weights; cls 0=M (interior w), 1=L (w=0), 2=R (w=127).
                # Output rows r0..r0+na-1 into asb columns [r0-2, r0-2+na).
                for cls in range(3):
                    for cot in range(2):
                        psa_t = psp.tile([128, 4, 128], F32, tag="ps2", name="ps_a", bufs=2)
                        mms = []
                        for cit in range(2):
                            for ky in range(3):
                                j = ((cls * 3 + ky) * 2 + cit) * 2 + cot
                                mms.append((wa[:, j, :],
                                            umask[cit][:, r0 - 1 + ky:r0 - 1 + ky + na]))
                        _mm_group(nc, psa_t[:, 0, 0:na], mms)
                        if asb[cot][cls] is None:
                            asb[cot][cls] = actp.tile([128, NA], F32, name=f"asb{cls}{cot}")
                        nc.scalar.copy(asb[cot][cls][:, r0 - 2:r0 - 2 + na],
                                       psa_t[:, 0, 0:na])
                # afull = A_M + bias_pc1 + pneg (per relu1 row);
                # afdL/afdR = A_L - A_M / A_R - A_M (w-edge fixups, pre-ReLU).
                for cot in range(2):
                    nc.vector.scalar_tensor_tensor(
                        afull[cot][:, r0:r0 + na], asb[cot][0][:, r0 - 2:r0 - 2 + na],
                        biases[:, 4 + cot:5 + cot], pnegb[:, r0:r0 + na],
                        op0=ALU.add, op1=ALU.add)
                    nc.vector.tensor_sub(afdl[cot][:, r0:r0 + na],
                                         asb[cot][1][:, r0 - 2:r0 - 2 + na],
                                         asb[cot][0][:, r0 - 2:r0 - 2 + na])
                    nc.vector.tensor_sub(afdr[cot][:, r0:r0 + na],
                                         asb[cot][2][:, r0 - 2:r0 - 2 + na],
                                         asb[cot][0][:, r0 - 2:r0 - 2 + na])

            def emit_b():
                # B_var(o,w): 1-D w-conv of dpad with ky-summed merge
                # weights; var 0=M (all ky), 1=ky0 only, 2=ky2 only.
                for var in range(3):
                    for cot in range(2):
                        psb_t = psp.tile([128, 4, 128], F32, tag="ps2", name="ps_b", bufs=2)
                        mms = []
                        for cit in range(2):
                            for kx in range(3):
                                j = ((var * 3 + kx) * 2 + cit) * 2 + cot
                                mms.append((wb[:, j, :], dpad[cit][:, kx:kx + W]))
                        _mm_group(nc, psb_t[:, 0, :], mms)
                        t = actp.tile([128, 128], F32, name=f"bt{var}{cot}")
                        # scalar, not vector: the vector queue is the serial
                        # resource in the post-up funnel
                        nc.scalar.copy(t[:, :], psb_t[:, 0, :])
                        bt[cot][var] = t

            def btaf_prep(s, nr, cot):
                # btaf[:, r, w] = bt_M[:, w] + afull[:, s+r] + edge fixups —
                # precomputed on vector OFF the c1->relu1 critical chain.
                t = osp.tile([128, 4, 128], F32, tag="btaf", name="btaf", bufs=6)
                bt_b = bt[cot][0][:, :].unsqueeze(1).broadcast_to([128, nr, 128])
                af_b = afull[cot][:, s:s + nr].unsqueeze(2).broadcast_to([128, nr, 128])
                nc.vector.tensor_add(t[:, 0:nr, :], bt_b, af_b)
                nc.vector.tensor_add(t[:, 0:nr, 0], t[:, 0:nr, 0], afdl[cot][:, s:s + nr])
                nc.vector.tensor_add(t[:, 0:nr, W - 1], t[:, 0:nr, W - 1], afdr[cot][:, s:s + nr])
                for r in range(nr):
                    sr = s + r
                    if sr == G:
                        nc.vector.scalar_tensor_tensor(
                            t[:, r, :], bt[cot][1][:, :], htopneg[:, sr:sr + 1],
                            t[:, r, :], op0=ALU.mult, op1=ALU.add)
                    if sr == HS - G - 1:
                        nc.vector.scalar_tensor_tensor(
                            t[:, r, :], bt[cot][2][:, :], hbotneg[:, sr:sr + 1],
                            t[:, r, :], op0=ALU.mult, op1=ALU.add)
                return t

            btafs = {}

            def emit_relu1_prep(s, nr):
                # emitted one step AHEAD of the mm stage so the vector queue
                # has the btaf ready when the c1 MM issues
                btafs[s] = [btaf_prep(s, nr, cot) for cot in range(2)]

            def emit_relu1_mms(s, nr, tag="ps", bufs=4):
                # relu1 = relu(c1(x) + A + B + bias): c1 MM -> STT -> ACT
                for cot in range(2):
                    btaf = btafs[s][cot]
                    ps = psp.tile([128, 4, 128], F32, tag=tag, name="ps_p", bufs=bufs)
                    nc.tensor.matmul(ps[:, 0:nr, :], wc1[:, :, cot, :],
                                     x8[:, :, s:s + nr, 1:W + 1],
                                     start=True, stop=True, perf_mode=DROW)
                    nc.vector.scalar_tensor_tensor(
                        ps[:, 0:nr, :], ps[:, 0:nr, :], qs["c1"][cot],
                        btaf[:, 0:nr, :], op0=ALU.mult, op1=ALU.add)
                    nc.scalar.activation(r1[cot][:, s:s + nr, 1:W + 1], ps[:, 0:nr, :],
                                         RELU, bias=0.0, scale=1.0)
                    nc.vector.tensor_copy(r1f8[:, cot, s:s + nr, 1:W + 1],
                                          r1[cot][:, s:s + nr, 1:W + 1])

            # up-conv blocks with merge work woven in.  Own rows are 3..66
            # (the halo rows 1,2,67,68 come from the exchange); the two
            # boundary blocks run first so the exchange slots are ready
            # before the collective fires.
            up_blocks = [63] + [3 + 4 * i for i in range(15)]
            for b, s in enumerate(up_blocks, start=1):
                for cot in range(2):
                    ps = psp.tile([128, 4, 128], F32, tag="ps", name="ps_up", bufs=4)
                    _mm_group(nc, ps[:, :, :], _conv3_mms8(wup, x8, s, 4, cot),
                              perf_mode=DROW)
                    nc.vector.reduce_max(uraw[cot][:, s:s + 4], ps[:, :, :], axis=AX_X)
                    nc.scalar.activation(ufin[cot][:, s:s + 4], uraw[cot][:, s:s + 4],
                                         RELU, bias=biases[:, cot:cot + 1],
                                         scale=qs["up"][cot])
                    nc.vector.tensor_mul(umask[cot][:, s:s + 4], ufin[cot][:, s:s + 4],
                                         hv[:, s:s + 4])
                if b == 2:
                    emit_cc_fill_and_reduce()
                elif b == 9:
                    with tc.high_priority():
                        emit_dfin_dpad()   # first collective done long before
                elif b == 10:
                    with tc.high_priority():
                        emit_uex()         # second (tiny) collective done
                elif b == 11:
                    emit_a_half(2, NA1)    # umask rows 1..42 available
                    with tc.high_priority():
                        emit_b()
                elif b in (12, 14, 16):
                    # a few early relu1 blocks, spread so the vector queue
                    # keeps pace with the up conv; preps go one up-block
                    # ahead of the matmul stage, and the woven c1 MMs use a
                    # dedicated psum tag so the up-conv bank rotation never
                    # couples to the relu1 STT chain
                    emit_relu1_prep(2 + 4 * ((b - 12) // 2), 4)
                elif b in (13, 15):
                    emit_relu1_mms(2 + 4 * ((b - 13) // 2), 4, tag="pse", bufs=2)

            emit_relu1_mms(10, 4, tag="pse", bufs=2)
            emit_relu1_prep(14, 4)

            # ---- c2 for the rows already assembled, then A half 2, then the
            # remaining relu1 and c2 blocks interleaved so the PE always has
            # a c2 block queued while relu1 chains run ----
            def emit_c2_block(s, nr):
                for cot in range(2):
                    ps = psp.tile([128, 4, 128], F32, tag="ps2", name="ps_c2", bufs=2)
                    # the (0,0) tap runs as one fp8 DoubleRow MM; the bf16
                    # weights are pre-scaled to match, and the common scale
                    # comes back out in the ACT below
                    nc.tensor.matmul(ps[:, 0:nr, :], wc2f8[:, :, cot, :],
                                     r1f8[:, :, s - 1:s - 1 + nr, 0:W],
                                     start=True, stop=False, perf_mode=DROW)
                    mms = _conv3_mms(wc2, r1, s, nr, cot, skip00=True)
                    for k, (lhsT, rhs) in enumerate(mms):
                        nc.tensor.matmul(ps[:, 0:nr, :], lhsT, rhs,
                                         start=False, stop=(k == len(mms) - 1))
                    ot = osp.tile([128, 4, 128], BF16, name="ot")
                    nc.scalar.activation(ot[:, 0:nr, :], ps[:, 0:nr, :], RELU,
                                         bias=biases[:, 6 + cot:7 + cot],
                                         scale=qs["c2"][cot])
                    if s >= G + 56:
                        # split the tail stores across rings so the last
                        # store's serial latency is halved
                        for r in range(0, nr, 2):
                            nc.sync.dma_start(out_d.ap()[cot, :, s - G + r:s - G + r + 2, :],
                                              ot[:, r:r + 2, :])
                    else:
                        nc.sync.dma_start(out_d.ap()[cot, :, s - G:s - G + nr, :], ot[:, 0:nr, :])

            emit_c2_block(G, 4)     # needs only relu1 rows 2..7 (done mid-up)
            emit_a_half(2 + NA1, NA2)

            r1_rest = [(14 + 4 * i, 4) for i in range(13)] + [(66, 2)]
            c2rest = [(G + 4 * (i + 1), 4) for i in range(14)] + [(G + 60, 2), (G + 62, 2)]
            for k in range(len(c2rest)):
                if k < len(r1_rest):
                    emit_relu1_mms(*r1_rest[k])
                if k + 1 < len(r1_rest):
                    emit_relu1_prep(*r1_rest[k + 1])
                emit_c2_block(*c2rest[k])

    nc.compile()
    return nc


def _pack3(w):
    # [256o, 256i, 3, 3] -> [128ci, j, 128co], j = ((ky*3+kx)*2+cit)*2+cot
    a = w.reshape(2, 128, 2, 128, 3, 3).transpose(3, 4, 5, 2, 0, 1)
    return np.ascontiguousarray(a.reshape(128, 36, 128)).astype(NP_BF16)


def _q8(a, s):
    return np.clip(a * s, -240.0, 240.0).astype(NP_FP8)


def _pack3_fp8(w, s_cot):
    # [256o, 256i, 3, 3] -> [2cit, 2cot, 128ci, j=ky*3+kx, 128co] fp8,
    # scaled per output-channel tile (cot)
    ws = w * np.repeat(s_cot, 128)[:, None, None, None]
    a = ws.reshape(2, 128, 2, 128, 3, 3).transpose(2, 0, 3, 4, 5, 1)
    # dims now [cit, cot, ci, ky, kx, co]
    return np.ascontiguousarray(_q8(a.reshape(2, 2, 128, 9, 128), 1.0))


def _pack1_fp8(w, s_cot):
    # [256o, 256i, 1, 1] -> [2cit, 128ci, cot, 128co] fp8
    ws = w[:, :, 0, 0] * np.repeat(s_cot, 128)[:, None]
    a = ws.reshape(2, 128, 2, 128).transpose(2, 3, 0, 1)
    return np.ascontiguousarray(_q8(a, 1.0))


def _pack_sep(wk3):
    # packs a [3var/cls, 3k, 256, 256] stack into [128ci, j, 128co],
    # j = ((v*3+k)*2+cit)*2+cot
    a = wk3.reshape(3, 3, 2, 128, 2, 128).transpose(5, 0, 1, 4, 2, 3)
    return np.ascontiguousarray(a.reshape(128, 36, 128)).astype(NP_BF16)


def _prep(inputs):
    x = np.asarray(inputs["x"], dtype=np.float32)

    fw, fb = {}, {}
    for n in ["up", "down", "p", "c1", "c2"]:
        g = np.asarray(inputs[f"g_{n}"], np.float32)
        v = np.asarray(inputs[f"v_{n}"], np.float32)
        m = np.asarray(inputs[f"m_{n}"], np.float32)
        b = np.asarray(inputs[f"b_{n}"], np.float32)
        w = np.asarray(inputs[f"w_{n}"], np.float32)
        s = g / np.sqrt(v + EPS)
        fw[n] = w * s[:, None, None, None]
        fb[n] = b - m * s

    sx = QCLIP / max(np.abs(x).max(), 1e-30)

    def wscale(w):
        m2 = np.abs(w).reshape(2, -1).max(axis=1)
        return QCLIP / np.maximum(m2, 1e-30)

    swup, swdn, swc1 = wscale(fw["up"]), wscale(fw["down"]), wscale(fw["c1"])
    swc2 = wscale(fw["c2"])
    qs = {
        "up": [float(1.0 / (sx * swup[t])) for t in range(2)],
        "dn": [float(1.0 / (sx * swdn[t])) for t in range(2)],
        "c1": [float(1.0 / (sx * swc1[t])) for t in range(2)],
        "c2": [float(1.0 / swc2[t]) for t in range(2)],
    }

    wp = fw["p"]
    wa_stack = np.stack([
        np.stack([wp[:, :, ky, :].sum(-1) for ky in range(3)]),            # M
        np.stack([wp[:, :, ky, 1:].sum(-1) for ky in range(3)]),           # L (w=0)
        np.stack([wp[:, :, ky, :2].sum(-1) for ky in range(3)]),           # R (w=127)
    ])
    wb_stack = np.stack([
        np.stack([wp[:, :, :, kx].sum(-1) for kx in range(3)]),            # M
        np.stack([wp[:, :, 0, kx] for kx in range(3)]),                    # ky=0
        np.stack([wp[:, :, 2, kx] for kx in range(3)]),                    # ky=2
    ])
    consts = {
        "wup": _pack3_fp8(fw["up"], swup),
        "wdn": _pack3_fp8(fw["down"], swdn),
        "wc1": _pack1_fp8(fw["c1"], swc1),
        # c2: all taps bf16 pre-scaled by swc2 (scale comes back out in the
        # final ACT), except the (0,0) tap which runs fp8 DoubleRow; the fp8
        # r1 operand is quantized at scale 1.0 (values << 240)
        "wc2": _pack3(fw["c2"] * np.repeat(swc2, 128)[:, None, None, None]),
        "wc2f8": _pack1_fp8(fw["c2"][:, :, 0:1, 0:1], swc2),
        "wa": _pack_sep(wa_stack),
        "wb": _pack_sep(wb_stack),
    }
    bias_np = np.zeros((128, 8), np.float32)
    for k, arr in enumerate([fb["up"], fb["down"], fb["p"] + fb["c1"], fb["c2"]]):
        m2 = arr.reshape(2, 128)
        bias_np[:, 2 * k] = m2[0]
        bias_np[:, 2 * k + 1] = m2[1]
    consts["biases"] = bias_np

    def _bcast(row):
        return np.ascontiguousarray(
            np.broadcast_to(row.astype(np.float32)[None, :], (128, HS)))

    in_maps = []
    for core in range(N_CORES):
        b_i, half = core // 2, core % 2
        slab = np.zeros((256, HS, WPX), np.float32)
        if half == 0:
            slab[:, G:, 1:W + 1] = x[b_i][:, 0:HS - G, :]
            hv_row = (np.arange(HS) >= G)
            top_s, bot_s = G, None            # slab row of global row 0
        else:
            slab[:, :HS - G, 1:W + 1] = x[b_i][:, H - (HS - G):H, :]
            hv_row = (np.arange(HS) <= HS - G - 1)
            top_s, bot_s = None, HS - G - 1   # slab row of global row H-1
        x8 = np.ascontiguousarray(
            _q8(slab, sx).reshape(2, 128, HS, WPX).transpose(1, 0, 2, 3))
        pneg_row = np.where(hv_row, 0.0, NEG)
        htop_row = np.zeros(HS)
        if top_s is not None:
            htop_row[top_s] = -1.0
        hbot_row = np.zeros(HS)
        if bot_s is not None:
            hbot_row[bot_s] = -1.0
        # u-halo exchange: slots [g62,g63,g64,g65]; half 0 owns the first
        # two, half 1 the last two
        sm_row = np.array([1., 1., 0., 0.] if half == 0 else [0., 0., 1., 1.],
                          np.float32)
        sb_row = np.where(sm_row > 0, 0.0, -3e38).astype(np.float32)
        in_maps.append({
            "x8": x8, "hv": _bcast(hv_row), "pnegb": _bcast(pneg_row),
            "htopneg": _bcast(htop_row), "hbotneg": _bcast(hbot_row),
            "sendm": np.ascontiguousarray(np.broadcast_to(sm_row, (128, 4))),
            "sendb": np.ascontiguousarray(np.broadcast_to(sb_row, (128, 4))),
            **consts})
    return in_maps, qs


def _run(inputs, trace=False):
    # Build a fresh Bass program per call: re-executing an already-loaded
    # NEFF on these cores intermittently trips NRT_EXEC_UNIT_UNRECOVERABLE,
    # while a fresh build+load is reliable (neuronxcc cache keeps it fast).
    in_maps, qs = _prep(inputs)
    nc = _build(qs)
    res = bass_utils.run_bass_kernel_spmd(
        nc, in_maps, core_ids=list(range(N_CORES)), trace=trace)
    out = np.empty((B, C, H, W), np.float32)
    for core in range(N_CORES):
        b_i, half = core // 2, core % 2
        r = np.asarray(res.results[core]["out"]).reshape(256, 64, W)
        out[b_i, :, half * 64:(half + 1) * 64, :] = r.astype(np.float32)
    return out, res


def kernel(**inputs) -> np.ndarray:
    out, _ = _run(inputs, trace=False)
    return out
